# revision 1
# baseline (speedup 1.0000x reference)
"""Causal attention (B=4, Sq=Sk=2048, D=1024, f32) on 8 TRN2 NeuronCores.

Strategy: pure data-parallel (no collectives). Each core handles one
(batch, half) shard: batch b = core//2, and half of the query rows of
that batch, chosen as an interleaving of 128-row tiles that balances
the causal workload. All 8 cores run the same program (SPMD); per-core
variation (which query rows, causal mask offsets) is carried entirely
in the data.

Per-core schedule: 8 query tiles of 128 rows, slot s covering keys
[0, 256*(s+1)).  A core's 8 query tiles are assigned to slots so that
each tile's causal need (gq+128 keys) fits its slot.  The causal
boundary is applied with an additive -1e9 mask (host-computed per slot)
on the final key tile of each slot.

Compute: S = Q K^T via float32r matmuls (tf32-class precision, 1
cycle/row) on host-pre-transposed Q/K layouts; softmax without
max-subtraction (logits S/32 ~ N(0,1), exp is safe) with the row-sum
fused into the exp activation (accum_out); P cast to bf16 by the exp;
P^T via DMA-transpose (bf16); P^T V accumulated over all key chunks in
PSUM; final 1/rowsum scaling on the way out.
"""

import os
import numpy as np
import ml_dtypes

B, SQ, SK, D = 4, 2048, 2048, 1024
NCORES = 8
P = 128                      # partitions / tile rows
NDC = D // P                 # 8 d-chunks of 128
NKC = SK // P                # 16 k-chunks of 128
KTILE = 512                  # key tile (free dim of S matmul)
NSLOT = 8                    # query tiles per core
SLOT_KLEN = [256 * (s + 1) for s in range(NSLOT)]   # keys covered per slot
# query-tile (128-row) indices of the batch handled by core parity j,
# ordered by slot (ascending causal need); complement pairs sum equally.
TILES_J0 = [0, 3, 5, 6, 8, 11, 13, 14]
TILES_J1 = [1, 2, 4, 7, 9, 10, 12, 15]
NEG = -1.0e9
SCALE = 1.0 / 32.0           # 1/sqrt(D)

_CACHE = {}


def _build_nc():
    import concourse.bacc as bacc
    import concourse.tile as tile
    import concourse.mybir as mybir

    dt = mybir.dt
    nc = bacc.Bacc("TRN2", target_bir_lowering=False, debug=False,
                   num_devices=NCORES)

    qt_ext = nc.dram_tensor("qt", [NDC, P, NSLOT * P], dt.float32r,
                            kind="ExternalInput").ap()
    kt_ext = nc.dram_tensor("kt", [NDC, P, SK], dt.float32r,
                            kind="ExternalInput").ap()
    v_ext = nc.dram_tensor("v", [NKC, P, D], dt.bfloat16,
                           kind="ExternalInput").ap()
    m_ext = nc.dram_tensor("maskneg", [NSLOT, P, KTILE], dt.float32,
                           kind="ExternalInput").ap()
    out_ext = nc.dram_tensor("out", [NSLOT * P, D], dt.float32,
                             kind="ExternalOutput").ap()

    # flat stage list: (slot, k-tile index, k0, kw, first, last)
    stages = []
    for s in range(NSLOT):
        klen = SLOT_KLEN[s]
        nk = (klen + KTILE - 1) // KTILE
        for kt in range(nk):
            k0 = kt * KTILE
            kw = min(KTILE, klen - k0)
            stages.append((s, kt, k0, kw, kt == 0, kt == nk - 1))

    with tile.TileContext(nc) as tc:
        with tc.tile_pool(name="big", bufs=1) as big, \
             tc.tile_pool(name="work", bufs=3) as work, \
             tc.tile_pool(name="acc", bufs=2) as acc, \
             tc.tile_pool(name="spsum", bufs=2, space="PSUM") as spsum, \
             tc.tile_pool(name="opsum", bufs=2, space="PSUM") as opsum:

            qt_sb = big.tile([P, NDC, NSLOT * P], dt.float32r)
            kt_sb = big.tile([P, NDC, SK], dt.float32r)
            v_sb = big.tile([P, NKC, D], dt.bfloat16)
            mask_sb = big.tile([P, NSLOT, KTILE], dt.float32)

            for s in range(NSLOT):
                nc.sync.dma_start(mask_sb[:, s], m_ext[s])
            for c in range(NDC):
                nc.sync.dma_start(qt_sb[:, c], qt_ext[c])

            kt_loaded = 0            # prefix of keys loaded (multiple of 256)

            def load_keys(klen):
                nonlocal kt_loaded
                if klen <= kt_loaded:
                    return
                for c in range(NDC):
                    nc.sync.dma_start(kt_sb[:, c, kt_loaded:klen],
                                      kt_ext[c, :, kt_loaded:klen])
                for kc in range(kt_loaded // P, klen // P):
                    nc.sync.dma_start(v_sb[:, kc], v_ext[kc])
                kt_loaded = klen

            # state carried between pipeline stages
            prev = None              # (s, kt, k0, kw, pt_tile, last)
            o_ps = None
            rsums = None

            def emit_pv(stage):
                s, kt, k0, kw, pt_t, last = stage
                nch = kw // P
                for c in range(nch):
                    kc = k0 // P + c
                    first_mm = (kt == 0 and c == 0)
                    last_mm = (last and c == nch - 1)
                    for h in range(2):
                        nc.tensor.matmul(
                            o_ps[:, h * KTILE:(h + 1) * KTILE],
                            pt_t[:, c],
                            v_sb[:, kc, h * KTILE:(h + 1) * KTILE],
                            start=first_mm, stop=last_mm)

            def finish_slot(s):
                nk = (SLOT_KLEN[s] + KTILE - 1) // KTILE
                rtot = work.tile([P, 1], dt.float32, tag="rtot")
                nc.vector.tensor_reduce(rtot[:], rsums[:, :nk],
                                        axis=mybir.AxisListType.X,
                                        op=mybir.AluOpType.add)
                recip = work.tile([P, 1], dt.float32, tag="recip")
                nc.vector.reciprocal(recip[:], rtot[:])
                o_sb = acc.tile([P, D], dt.float32, tag="o_sb")
                nc.vector.tensor_scalar(o_sb[:], o_ps[:], recip[:], None,
                                        op0=mybir.AluOpType.mult)
                nc.sync.dma_start(out_ext[s * P:(s + 1) * P, :], o_sb[:])

            for s, kt, k0, kw, first, last in stages:
                load_keys(max(k0 + kw, 512 if s == 0 and kt == 0 else 0))
                if first:
                    if s > 0:
                        # retire previous slot's PV before switching o_ps
                        if prev is not None and prev[0] == s - 1:
                            emit_pv(prev)
                            prev = None
                            finish_slot(s - 1)
                    o_ps = opsum.tile([P, D], dt.float32, tag="o")
                    rsums = acc.tile([P, 4], dt.float32, tag="rsums")

                # S = (Q K^T) for this tile: accumulate 8 d-chunks
                s_ps = spsum.tile([P, KTILE], dt.float32, tag="s")
                q0 = s * P
                for c in range(NDC):
                    nc.tensor.matmul(s_ps[:, :kw],
                                     qt_sb[:, c, q0:q0 + P],
                                     kt_sb[:, c, k0:k0 + kw],
                                     start=(c == 0), stop=(c == NDC - 1))
                if last:
                    nc.vector.tensor_tensor(s_ps[:, :kw], s_ps[:, :kw],
                                            mask_sb[:, s, :kw],
                                            op=mybir.AluOpType.add)
                # P = exp(S/32), rowsum fused
                p_t = work.tile([P, KTILE], dt.bfloat16, tag="p")
                nc.scalar.activation(p_t[:, :kw], s_ps[:, :kw],
                                     mybir.ActivationFunctionType.Exp,
                                     scale=SCALE,
                                     accum_out=rsums[:, kt:kt + 1])
                # P^T chunks via DMA transpose (on ACT's DGE queue, away
                # from the load/store queue on sync)
                pt_t = work.tile([P, KTILE // P, P], dt.bfloat16, tag="pt")
                for c in range(kw // P):
                    nc.scalar.dma_start_transpose(
                        pt_t[:, c], p_t[:, c * P:(c + 1) * P])

                if prev is not None:
                    emit_pv(prev)
                prev = (s, kt, k0, kw, pt_t, last)

            emit_pv(prev)
            finish_slot(NSLOT - 1)

    nc.compile()
    return nc


def _get_nc():
    if "nc" not in _CACHE:
        os.environ.setdefault("JAX_COMPILATION_CACHE_DIR", "/tmp/jax_comp_cache")
        try:
            import jax
            jax.config.update("jax_compilation_cache_dir", "/tmp/jax_comp_cache")
            jax.config.update("jax_persistent_cache_min_entry_size_bytes", -1)
            jax.config.update("jax_persistent_cache_min_compile_time_secs", 0)
        except Exception:
            pass
        _CACHE["nc"] = _build_nc()
    return _CACHE["nc"]


def _host_masks(tiles):
    """[NSLOT, 128, KTILE] additive mask for the final key-tile of each slot."""
    masks = np.zeros((NSLOT, P, KTILE), np.float32)
    pp = np.arange(P)[:, None]
    for s in range(NSLOT):
        gq = P * tiles[s]
        klen = SLOT_KLEN[s]
        k0_last = ((klen + KTILE - 1) // KTILE - 1) * KTILE
        kw = klen - k0_last
        kk = k0_last + np.arange(kw)[None, :]
        masks[s, :, :kw] = np.where(kk <= gq + pp, 0.0, NEG)
    return masks


def make_in_maps(query, key, value):
    query = np.asarray(query, np.float32)
    key = np.asarray(key, np.float32)
    value = np.asarray(value, np.float32)
    in_maps = []
    for core in range(NCORES):
        b, j = divmod(core, 2)
        tiles = TILES_J0 if j == 0 else TILES_J1
        qrows = np.concatenate([query[b, P * t:P * (t + 1)] for t in tiles])
        qt = np.ascontiguousarray(qrows.T).reshape(NDC, P, NSLOT * P)
        kt = np.ascontiguousarray(key[b].T).reshape(NDC, P, SK)
        v = value[b].astype(ml_dtypes.bfloat16).reshape(NKC, P, D)
        in_maps.append({
            "qt": qt,
            "kt": kt,
            "v": v,
            "maskneg": _host_masks(tiles),
        })
    return in_maps


def assemble(results):
    out = np.empty((B, SQ, D), np.float32)
    for core in range(NCORES):
        b, j = divmod(core, 2)
        tiles = TILES_J0 if j == 0 else TILES_J1
        o = results[core]["out"]
        for s, t in enumerate(tiles):
            out[b, P * t:P * (t + 1)] = o[P * s:P * (s + 1)]
    return out


def kernel(query, key, value, _run_kwargs=None):
    from concourse.bass_utils import run_bass_kernel_spmd
    nc = _get_nc()
    in_maps = make_in_maps(query, key, value)
    kw = dict(_run_kwargs or {})
    res = run_bass_kernel_spmd(nc, in_maps, list(range(NCORES)), **kw)
    out = assemble(res.results)
    if _run_kwargs is not None:
        _CACHE["last_result"] = res
    return out


# revision 4
# speedup vs baseline: 2.3683x; 2.3683x over previous
"""Causal attention (B=4, Sq=Sk=2048, D=1024, f32) on 8 TRN2 NeuronCores.

Strategy: pure data-parallel (no collectives). Each core handles one
(batch, half) shard: batch b = core//2, and half of the query rows of
that batch, chosen as an interleaving of 128-row tiles that balances
the causal workload. All 8 cores run the same program (SPMD); per-core
variation (which query rows, causal mask offsets) is carried entirely
in the data.

Per-core schedule: 8 query tiles of 128 rows, slot s covering keys
[0, 256*(s+1)).  A core's 8 query tiles are assigned to slots so that
each tile's causal need (gq+128 keys) fits its slot.  The causal
boundary is applied with an additive -1e9 mask (host-computed per slot)
on the final key tile of each slot.

Compute: S = Q K^T via float32r matmuls (tf32-class precision, ~1
cycle/row) on host-pre-transposed Q/K layouts; softmax without
max-subtraction (logits S/32 ~ N(0,1), exp is safe) with the row-sum
fused into the exp activation (accum_out); P cast to bf16 by the exp;
P^T via TensorE transpose (keeps the PE stream dense so the HAM clock
gate stays at 2.4 GHz — DMA-transpose latency starved the PE in v1);
P^T V accumulated over all key chunks in PSUM; final 1/rowsum scaling
on the way out.  The (S, exp, transpose, PV) chain is software-
pipelined two stages deep so the PE never waits on ACT/DVE.
"""

import os
import numpy as np
import ml_dtypes

B, SQ, SK, D = 4, 2048, 2048, 1024
NCORES = 8
P = 128                      # partitions / tile rows
NDC = D // P                 # 8 d-chunks of 128
NKC = SK // P                # 16 k-chunks of 128
KTILE = 512                  # key tile (free dim of S matmul)
NSLOT = 8                    # query tiles per core
SLOT_KLEN = [256 * (s + 1) for s in range(NSLOT)]   # keys covered per slot
# query-tile (128-row) indices of the batch handled by core parity j,
# ordered by slot (ascending causal need); complement pairs sum equally.
TILES_J0 = [0, 3, 5, 6, 8, 11, 13, 14]
TILES_J1 = [1, 2, 4, 7, 9, 10, 12, 15]
NEG = -1.0e9
SCALE = 1.0 / 32.0           # 1/sqrt(D)

_CACHE = {}


def _build_nc():
    import concourse.bacc as bacc
    import concourse.tile as tile
    import concourse.mybir as mybir
    from concourse.masks import make_identity

    dt = mybir.dt
    nc = bacc.Bacc("TRN2", target_bir_lowering=False, debug=False,
                   num_devices=NCORES)

    qt_ext = nc.dram_tensor("qt", [NDC, P, NSLOT * P], dt.float32r,
                            kind="ExternalInput").ap()
    kt_ext = nc.dram_tensor("kt", [NDC, P, SK], dt.float32r,
                            kind="ExternalInput").ap()
    v_ext = nc.dram_tensor("v", [NKC, P, D], dt.bfloat16,
                           kind="ExternalInput").ap()
    m_ext = nc.dram_tensor("maskneg", [NSLOT, P, KTILE], dt.float32,
                           kind="ExternalInput").ap()
    out_ext = nc.dram_tensor("out", [NSLOT * P, D], dt.float32,
                             kind="ExternalOutput").ap()

    # flat stage list: (slot, k-tile index, k0, kw, last)
    stages = []
    for s in range(NSLOT):
        klen = SLOT_KLEN[s]
        nk = (klen + KTILE - 1) // KTILE
        for kt in range(nk):
            k0 = kt * KTILE
            kw = min(KTILE, klen - k0)
            stages.append((s, kt, k0, kw, kt == nk - 1))

    with tile.TileContext(nc) as tc:
        with tc.tile_pool(name="big", bufs=1) as big, \
             tc.tile_pool(name="work", bufs=3) as work, \
             tc.tile_pool(name="acc", bufs=2) as acc, \
             tc.tile_pool(name="spsum", bufs=2, space="PSUM") as spsum, \
             tc.tile_pool(name="tpsum", bufs=2, space="PSUM") as tpsum, \
             tc.tile_pool(name="opsum", bufs=2, space="PSUM") as opsum:

            qt_sb = big.tile([P, NDC, NSLOT * P], dt.float32r)
            kt_sb = big.tile([P, NDC, SK], dt.float32r)
            v_sb = big.tile([P, NKC, D], dt.bfloat16)
            mask_sb = big.tile([P, NSLOT, KTILE], dt.float32)
            ident = big.tile([P, P], dt.bfloat16)
            make_identity(nc, ident[:])

            for s in range(NSLOT):
                nc.sync.dma_start(mask_sb[:, s], m_ext[s])
            for c in range(NDC):
                nc.sync.dma_start(qt_sb[:, c], qt_ext[c])

            kt_loaded = 0            # prefix of keys loaded (multiple of 512)

            def load_keys(klen):
                nonlocal kt_loaded
                klen = min(-(-klen // KTILE) * KTILE, SK)
                if klen <= kt_loaded:
                    return
                for c in range(NDC):
                    nc.sync.dma_start(kt_sb[:, c, kt_loaded:klen],
                                      kt_ext[c, :, kt_loaded:klen])
                for kc in range(kt_loaded // P, klen // P):
                    nc.sync.dma_start(v_sb[:, kc], v_ext[kc])
                kt_loaded = klen

            state = {}               # per-stage-index carried tiles

            def emit_s(i):
                s, kt, k0, kw, last = stages[i]
                load_keys(k0 + kw)
                if kt == 0:
                    state[("rsums", s)] = acc.tile([P, 4], dt.float32,
                                                   name=f"rsums{s}", tag="rsums")
                rsums = state[("rsums", s)]
                s_ps = spsum.tile([P, KTILE], dt.float32, tag="s")
                q0 = s * P
                for c in range(NDC):
                    nc.tensor.matmul(s_ps[:, :kw],
                                     qt_sb[:, c, q0:q0 + P],
                                     kt_sb[:, c, k0:k0 + kw],
                                     start=(c == 0), stop=(c == NDC - 1))
                if last:
                    nc.vector.tensor_tensor(s_ps[:, :kw], s_ps[:, :kw],
                                            mask_sb[:, s, :kw],
                                            op=mybir.AluOpType.add)
                p_t = work.tile([P, KTILE], dt.bfloat16, tag="p")
                nc.scalar.activation(p_t[:, :kw], s_ps[:, :kw],
                                     mybir.ActivationFunctionType.Exp,
                                     scale=SCALE,
                                     accum_out=rsums[:, kt:kt + 1])
                state[("p", i)] = p_t

            def emit_t(i):
                s, kt, k0, kw, last = stages[i]
                p_t = state.pop(("p", i))
                nch = kw // P
                pt_ps = tpsum.tile([P, KTILE // P, P], dt.bfloat16, tag="tp")
                for c in range(nch):
                    nc.tensor.transpose(pt_ps[:, c], p_t[:, c * P:(c + 1) * P],
                                        ident[:])
                pt_t = work.tile([P, KTILE // P, P], dt.bfloat16, tag="pt")
                nc.vector.tensor_copy(pt_t[:, :nch], pt_ps[:, :nch])
                state[("pt", i)] = pt_t

            def emit_pv(i):
                s, kt, k0, kw, last = stages[i]
                if kt == 0:
                    state[("o", s)] = opsum.tile([P, D], dt.float32, name=f"o{s}", tag="o")
                o_ps = state[("o", s)]
                pt_t = state.pop(("pt", i))
                nch = kw // P
                for c in range(nch):
                    kc = k0 // P + c
                    first_mm = (kt == 0 and c == 0)
                    last_mm = (last and c == nch - 1)
                    for h in range(2):
                        nc.tensor.matmul(
                            o_ps[:, h * KTILE:(h + 1) * KTILE],
                            pt_t[:, c],
                            v_sb[:, kc, h * KTILE:(h + 1) * KTILE],
                            start=first_mm, stop=last_mm)
                if last:
                    finish_slot(s)

            def finish_slot(s):
                nk = (SLOT_KLEN[s] + KTILE - 1) // KTILE
                o_ps = state.pop(("o", s))
                rsums = state.pop(("rsums", s))
                rtot = work.tile([P, 1], dt.float32, tag="rtot")
                nc.vector.tensor_reduce(rtot[:], rsums[:, :nk],
                                        axis=mybir.AxisListType.X,
                                        op=mybir.AluOpType.add)
                recip = work.tile([P, 1], dt.float32, tag="recip")
                nc.vector.reciprocal(recip[:], rtot[:])
                o_sb = acc.tile([P, D], dt.float32, tag="o_sb")
                nc.vector.tensor_scalar(o_sb[:], o_ps[:], recip[:], None,
                                        op0=mybir.AluOpType.mult)
                nc.sync.dma_start(out_ext[s * P:(s + 1) * P, :], o_sb[:])

            n = len(stages)
            for i in range(n + 2):
                if i < n:
                    emit_s(i)
                if 1 <= i <= n:
                    emit_t(i - 1)
                if i >= 2:
                    emit_pv(i - 2)

    nc.compile()
    return nc


def _get_nc():
    if "nc" not in _CACHE:
        os.environ.setdefault("JAX_COMPILATION_CACHE_DIR", "/tmp/jax_comp_cache")
        try:
            import jax
            jax.config.update("jax_compilation_cache_dir", "/tmp/jax_comp_cache")
            jax.config.update("jax_persistent_cache_min_entry_size_bytes", -1)
            jax.config.update("jax_persistent_cache_min_compile_time_secs", 0)
        except Exception:
            pass
        _CACHE["nc"] = _build_nc()
    return _CACHE["nc"]


def _host_masks(tiles):
    """[NSLOT, 128, KTILE] additive mask for the final key-tile of each slot."""
    masks = np.zeros((NSLOT, P, KTILE), np.float32)
    pp = np.arange(P)[:, None]
    for s in range(NSLOT):
        gq = P * tiles[s]
        klen = SLOT_KLEN[s]
        k0_last = ((klen + KTILE - 1) // KTILE - 1) * KTILE
        kw = klen - k0_last
        kk = k0_last + np.arange(kw)[None, :]
        masks[s, :, :kw] = np.where(kk <= gq + pp, 0.0, NEG)
    return masks


def make_in_maps(query, key, value):
    query = np.asarray(query, np.float32)
    key = np.asarray(key, np.float32)
    value = np.asarray(value, np.float32)
    in_maps = []
    for core in range(NCORES):
        b, j = divmod(core, 2)
        tiles = TILES_J0 if j == 0 else TILES_J1
        qrows = np.concatenate([query[b, P * t:P * (t + 1)] for t in tiles])
        qt = np.ascontiguousarray(qrows.T).reshape(NDC, P, NSLOT * P)
        kt = np.ascontiguousarray(key[b].T).reshape(NDC, P, SK)
        v = value[b].astype(ml_dtypes.bfloat16).reshape(NKC, P, D)
        in_maps.append({
            "qt": qt,
            "kt": kt,
            "v": v,
            "maskneg": _host_masks(tiles),
        })
    return in_maps


def assemble(results):
    out = np.empty((B, SQ, D), np.float32)
    for core in range(NCORES):
        b, j = divmod(core, 2)
        tiles = TILES_J0 if j == 0 else TILES_J1
        o = results[core]["out"]
        for s, t in enumerate(tiles):
            out[b, P * t:P * (t + 1)] = o[P * s:P * (s + 1)]
    return out


def kernel(query, key, value, _run_kwargs=None):
    from concourse.bass_utils import run_bass_kernel_spmd
    nc = _get_nc()
    in_maps = make_in_maps(query, key, value)
    kw = dict(_run_kwargs or {})
    res = run_bass_kernel_spmd(nc, in_maps, list(range(NCORES)), **kw)
    out = assemble(res.results)
    if _run_kwargs is not None:
        _CACHE["last_result"] = res
    return out
